# revision 53
# baseline (speedup 1.0000x reference)
"""KAN-GNN message passing on 8 TRN2 NeuronCores.

Strategy (data-parallel over nodes, per sharding hint):
 - Nodes are assigned to cores by a balanced 4-coloring: color c maps to the
   core pair {2c, 2c+1} and therefore to a contiguous 25088-row block of the
   AllGathered tables.  The coloring greedily splits every target's source
   list evenly across the 4 blocks, so the per-block slot rectangles stay
   near ceil(deg/4).  Within a color, nodes are dealt to its two cores by
   in-degree round-robin, keeping the per-core degree profiles matched.
 - Per core: KAN layer 1 on its node shard (3 fused matmuls per group of
   128 nodes; 4 groups share one PSUM bank so bias-add + relu run batched),
   then AllGather of the bf16 h1 table.
 - Aggregation: groups are packed into batches; per batch, 4 dma_gather
   calls (one per 25088-row table block, int16 indices relative to the
   block) pull all (target, slot) source rows into SBUF side by side; a
   halving tree of strided DVE adds reduces each block's slots for all
   groups at once, 3 adds combine the block partials, scale by 1/deg,
   PE-transpose, KAN layer 2, AllGather of the f32 h2 table (f32 because
   dma_gather rows must be a multiple of 256B), second batched
   gather/reduce, then a batched log_softmax (Exp/Ln tables load once).
 - All indices/permutations are precomputed on the host and baked into the
   (single, SPMD) program; per-core data goes in as input tensors.
"""
import numpy as np
import ml_dtypes

import concourse.bacc as bacc
import concourse.mybir as mybir
import concourse.tile as tile
import concourse.bass as bass
from concourse.bass_utils import run_bass_kernel_spmd

N_NODES = 100000
N_EDGES = 1600000
IN_F, HID_F, OUT_F = 128, 128, 64
K = 8               # cores
P = 128             # partitions / targets per group
J = 12544           # local nodes per core (98 * 128), 12500 real + 44 pad
G = J // P          # 98 groups
JREAL = N_NODES // K  # 12500
TBL = K * J         # 100352 rows in the all-gathered tables
NC_COL = 4          # colors = table blocks = core pairs
BLK = 2 * J         # 25088 rows per block (int16-addressable)
CREAL = 2 * JREAL   # real nodes per color
PAD_REL = JREAL     # block-relative all-zero row (first core's pad area)

RCAP = 40           # max padded slots (ng * D_R) per range-block per batch
NGMAX = 8           # max groups per batch
NB1 = 4             # phase-1 groups per PSUM bank block

BF16 = mybir.dt.bfloat16
F32 = mybir.dt.float32
I16 = mybir.dt.int16


def _color_sources(src, tgt, deg):
    """Greedy balanced 4-coloring of nodes: minimize per-target color skew.

    Returns colors[n] in 0..3 with exactly CREAL nodes per color.
    """
    eorder = np.argsort(src, kind="stable")
    tgt_by_src = tgt[eorder]
    indptr = np.zeros(N_NODES + 1, dtype=np.int64)
    np.cumsum(np.bincount(src, minlength=N_NODES), out=indptr[1:])

    cnt = np.zeros((N_NODES, NC_COL), dtype=np.int32)
    quota = np.ceil(np.maximum(deg, 1) / NC_COL).astype(np.int32)
    colors = np.full(N_NODES, -1, dtype=np.int8)
    sizes = np.zeros(NC_COL, dtype=np.int64)
    # process high-out-degree nodes first
    odeg = indptr[1:] - indptr[:-1]
    proc = np.argsort(-odeg, kind="stable")
    big = np.int64(1) << 40
    for n in proc:
        t = tgt_by_src[indptr[n]:indptr[n + 1]]
        ct = cnt[t, :]
        # hard penalty for pushing any target past its per-color quota
        cost = (ct >= quota[t][:, None]).sum(axis=0) * 10000 + ct.sum(axis=0)
        cost = cost + np.where(sizes >= CREAL, big, 0)
        c = int(np.argmin(cost))
        colors[n] = c
        sizes[c] += 1
        np.add.at(cnt[:, c], t, 1)
    # refinement passes: re-greedy each node with warm counts (size-neutral
    # moves only happen when strictly better, tracking exact sizes)
    for _ in range(2):
        for n in proc:
            t = tgt_by_src[indptr[n]:indptr[n + 1]]
            c0 = colors[n]
            np.add.at(cnt[:, c0], t, -1)
            sizes[c0] -= 1
            ct = cnt[t, :]
            cost = ((ct >= quota[t][:, None]).sum(axis=0) * 10000
                    + (ct >= (quota[t][:, None] + 1)).sum(axis=0) * 100000
                    + ct.sum(axis=0))
            cost = cost + np.where(sizes >= CREAL, big, 0)
            c = int(np.argmin(cost))
            colors[n] = c
            sizes[c] += 1
            np.add.at(cnt[:, c], t, 1)
    return colors


def _host_prep(x, edge_index, w1, b1, c1, w2, b2, c2):
    src = np.asarray(edge_index[0], dtype=np.int64)
    tgt = np.asarray(edge_index[1], dtype=np.int64)
    x = np.asarray(x, dtype=np.float32)

    deg = np.bincount(tgt, minlength=N_NODES)
    colors = _color_sources(src, tgt, deg)

    # within each color: in-degree-sorted round-robin onto its two cores
    core_of_node = np.empty(N_NODES, dtype=np.int64)
    j_of_node = np.empty(N_NODES, dtype=np.int64)
    for c in range(NC_COL):
        nodes_c = np.where(colors == c)[0]
        nodes_c = nodes_c[np.argsort(-deg[nodes_c], kind="stable")]
        core_of_node[nodes_c] = 2 * c + (np.arange(len(nodes_c)) % 2)
        j_of_node[nodes_c] = np.arange(len(nodes_c)) // 2
    pos_of_node = core_of_node * J + j_of_node

    degs_kj = np.zeros((K, J), dtype=np.int64)
    degs_kj[core_of_node, j_of_node] = deg

    # per-(group, color) slot rectangle heights, shared across cores
    ek = core_of_node[tgt]
    ej = j_of_node[tgt]
    ecol = colors[src].astype(np.int64)
    gs_all = ej // P
    cnt_gc = np.zeros((K, G, P, NC_COL), dtype=np.int64)
    np.add.at(cnt_gc, (ek, gs_all, ej % P, ecol), 1)
    Dgr = np.maximum(cnt_gc.max(axis=(0, 2)), 1)      # [G, NC_COL]

    # pack consecutive groups into batches with common per-color heights
    batches = []        # dicts: ga, ng, Ds[4], slot_off (gt cols), idx_off
    icol_off = 0
    g = 0
    while g < G:
        ng = 1
        Ds = Dgr[g].copy()
        while g + ng < G and ng < NGMAX:
            nD = np.maximum(Ds, Dgr[g + ng])
            if (ng + 1) * int(nD.max()) > RCAP:
                break
            Ds = nD
            ng += 1
        batches.append({
            "ga": g, "ng": ng, "Ds": [int(d) for d in Ds],
            "icol": icol_off,
        })
        icol_off += 8 * ng * int(Ds.sum())   # int16 cols (= rows/16)
        g += ng
    TOTC = icol_off
    for bt in batches:
        bt["icol2"] = TOTC + bt["icol"]

    # index tensor [K][128, 2*TOTC] int16 (wrap-16, replicated across 8 bands)
    # edge slot positions: sort edges by (core, target j, color) to rank them
    key = ((ek * J) + ej) * NC_COL + ecol
    eorder = np.argsort(key, kind="stable")
    skey = key[eorder]
    _, counts = np.unique(skey, return_counts=True)
    run_starts = np.concatenate([[0], np.cumsum(counts)[:-1]])
    d_in_run = np.arange(len(skey)) - np.repeat(run_starts, counts)
    eks = skey // (J * NC_COL)
    ejs = (skey // NC_COL) % J
    ecs = skey % NC_COL
    egs = ejs // P
    eps = ejs % P
    rel = (pos_of_node[src[eorder]] - ecs * BLK).astype(np.int64)
    assert (rel >= 0).all() and (rel < BLK).all()

    ga_of_g = np.zeros(G, dtype=np.int64)
    for bi, bt in enumerate(batches):
        for gg in range(bt["ng"]):
            ga_of_g[bt["ga"] + gg] = bi
    # flat list position within the batch's idx block for each edge:
    # ranges laid out color-major: color R block has ng*Ds[R] columns of 128.
    bts = batches
    b_of_e = ga_of_g[egs]
    icol_b = np.array([bt["icol"] for bt in bts], dtype=np.int64)
    ng_b = np.array([bt["ng"] for bt in bts], dtype=np.int64)
    ga_b = np.array([bt["ga"] for bt in bts], dtype=np.int64)
    Ds_b = np.array([bt["Ds"] for bt in bts], dtype=np.int64)       # [B, 4]
    DsCum_b = np.concatenate([np.zeros((len(bts), 1), np.int64),
                              np.cumsum(Ds_b, axis=1)], axis=1)     # [B, 5]
    # column within gather dest for this edge's slot (color-major blocks):
    gg_e = egs - ga_b[b_of_e]
    cc = ng_b[b_of_e] * DsCum_b[b_of_e, ecs] + gg_e * Ds_b[b_of_e, ecs] + d_in_run
    jlist = cc * P + eps                   # position in the batch's flat list
    # phase-2 (transpose-mode) section: per (batch, range) the list is
    # (group, target, slot)-major so gathered columns form per-target runs.
    icol2_b = TOTC + icol_b                      # phase-2 block start per batch
    TOTC2 = 2 * TOTC
    jlist2 = (gg_e * P + eps) * Ds_b[b_of_e, ecs] + d_in_run

    # band 0 (partitions 0-15): wrap-16 layout; pads -> PAD_REL (zero row)
    idx_all = np.zeros((K, P, TOTC2), dtype=np.int16)
    col16 = icol_b[b_of_e] + jlist // 16
    row16 = jlist % 16
    col16b = icol2_b[b_of_e] + (ng_b[b_of_e] * DsCum_b[b_of_e, ecs] * P
                                + jlist2) // 16
    row16b = (ng_b[b_of_e] * DsCum_b[b_of_e, ecs] * P + jlist2) % 16
    idx_all[:, :16, :] = PAD_REL
    idx_all[eks, row16, col16] = rel.astype(np.int16)
    idx_all[eks, row16b, col16b] = rel.astype(np.int16)
    # bands 1..7 are exact replicas of band 0
    for band in range(1, 8):
        idx_all[:, 16 * band:16 * (band + 1), :] = idx_all[:, :16, :]

    # per-core 1/deg  [K, P, G]  (0 for pad targets)
    with np.errstate(divide="ignore"):
        dr = 1.0 / np.maximum(degs_kj, 1).astype(np.float32)
    real = np.zeros((K, J), dtype=np.float32)
    real[:, :JREAL] = 1.0
    degrecip = (dr * np.where(real > 0, 1.0, 0.0)).reshape(K, G, P).transpose(0, 2, 1).copy()

    # pad-node mask [P, G] (same on every core)
    mask_j = (np.arange(J) < JREAL).astype(np.float32)
    mask_pg = mask_j.reshape(G, P).T.copy()

    # xT shards, bf16 [K][IN_F, J]
    xT = np.zeros((K, IN_F, J), dtype=ml_dtypes.bfloat16)
    for k in range(K):
        nodes_k = np.where(core_of_node == k)[0]
        xT[k][:, j_of_node[nodes_k]] = x[nodes_k].T.astype(ml_dtypes.bfloat16)

    # fused KAN weights
    A1 = (w1 + 0.1 * c1[:, :, 0]).astype(ml_dtypes.bfloat16)
    B1 = (0.1 * c1[:, :, 1]).astype(ml_dtypes.bfloat16)
    C1 = (0.1 * c1[:, :, 2]).astype(ml_dtypes.bfloat16)
    A2 = (w2 + 0.1 * c2[:, :, 0]).astype(ml_dtypes.bfloat16)
    B2 = (0.1 * c2[:, :, 1]).astype(ml_dtypes.bfloat16)
    C2 = (0.1 * c2[:, :, 2]).astype(ml_dtypes.bfloat16)
    b1b = np.tile(np.asarray(b1, np.float32)[None, :], (P, 1))
    b2b = np.tile(np.asarray(b2, np.float32)[None, :], (P, 1))
    ident = np.eye(P, dtype=np.float32)

    in_maps = []
    for k in range(K):
        in_maps.append({
            "xT": xT[k],
            "idx": idx_all[k],
            "degrecip": degrecip[k],
            "degrecip2": degrecip[k] ** 2,
            "degrecip3": degrecip[k] ** 3,
            "mask": mask_pg,
            "A1": A1, "B1": B1, "C1": C1,
            "A2": A2, "B2": B2, "C2": C2,
            "b1b": b1b, "b2b": b2b, "ident": ident,
            "identb": np.eye(P, dtype=ml_dtypes.bfloat16),
        })
    meta = {"batches": batches, "TOTC": TOTC2,
            "core_of_node": core_of_node, "j_of_node": j_of_node}
    return in_maps, meta


def build_program(batches, TOTC, dump=False):
    nc = bacc.Bacc("TRN2", target_bir_lowering=False, debug=False, num_devices=K,
                   dynamic_dma_scratch_size=24576, num_swdge_queues=4)

    xT = nc.dram_tensor("xT", [IN_F, J], BF16, kind="ExternalInput")
    idx = nc.dram_tensor("idx", [P, TOTC], I16, kind="ExternalInput")
    degrecip = nc.dram_tensor("degrecip", [P, G], F32, kind="ExternalInput")
    degrecip2 = nc.dram_tensor("degrecip2", [P, G], F32, kind="ExternalInput")
    degrecip3 = nc.dram_tensor("degrecip3", [P, G], F32, kind="ExternalInput")
    mask = nc.dram_tensor("mask", [P, G], F32, kind="ExternalInput")
    A1 = nc.dram_tensor("A1", [IN_F, HID_F], BF16, kind="ExternalInput")
    B1 = nc.dram_tensor("B1", [IN_F, HID_F], BF16, kind="ExternalInput")
    C1 = nc.dram_tensor("C1", [IN_F, HID_F], BF16, kind="ExternalInput")
    A2 = nc.dram_tensor("A2", [HID_F, OUT_F], BF16, kind="ExternalInput")
    B2 = nc.dram_tensor("B2", [HID_F, OUT_F], BF16, kind="ExternalInput")
    C2 = nc.dram_tensor("C2", [HID_F, OUT_F], BF16, kind="ExternalInput")
    b1b = nc.dram_tensor("b1b", [P, HID_F], F32, kind="ExternalInput")
    b2b = nc.dram_tensor("b2b", [P, OUT_F], F32, kind="ExternalInput")
    ident = nc.dram_tensor("ident", [P, P], F32, kind="ExternalInput")
    identb = nc.dram_tensor("identb", [P, P], BF16, kind="ExternalInput")
    y = nc.dram_tensor("y", [J, OUT_F], F32, kind="ExternalOutput")
    if dump:
        h1o = nc.dram_tensor("h1o", [J, HID_F], BF16, kind="ExternalOutput")
        h2o = nc.dram_tensor("h2o", [J, OUT_F], F32, kind="ExternalOutput")
        rawo = nc.dram_tensor("rawo", [HID_F, J], F32, kind="ExternalOutput")

    h1_in = nc.dram_tensor("h1_in", [J, HID_F], BF16, kind="Internal")
    h1_tbl = nc.dram_tensor("h1_tbl", [TBL, HID_F], BF16, kind="Internal",
                            addr_space="Shared")
    h2_in = nc.dram_tensor("h2_in", [J, OUT_F], F32, kind="Internal")
    h2_tbl = nc.dram_tensor("h2_tbl", [TBL, OUT_F], F32, kind="Internal",
                            addr_space="Shared")

    with tile.TileContext(nc) as tc:
        with (
            tc.tile_pool(name="consts", bufs=1) as cpool,
            tc.tile_pool(name="p1", bufs=2) as p1pool,
            tc.tile_pool(name="work", bufs=2) as wpool,
            tc.tile_pool(name="g1p", bufs=2) as g1pool,
            tc.tile_pool(name="idxp", bufs=2) as ipool,
            tc.tile_pool(name="psum", bufs=2, space="PSUM") as ppool,
            tc.tile_pool(name="psum2", bufs=3, space="PSUM") as ppool2,
        ):
            c_dr = cpool.tile([P, G], F32, tag="dr")
            nc.sync.dma_start(out=c_dr[:], in_=degrecip[:, :])
            c_dr2 = cpool.tile([P, G], F32, tag="dr2")
            nc.sync.dma_start(out=c_dr2[:], in_=degrecip2[:, :])
            c_dr3 = cpool.tile([P, G], F32, tag="dr3")
            nc.sync.dma_start(out=c_dr3[:], in_=degrecip3[:, :])
            c_mask = cpool.tile([P, G], F32, tag="mask")
            nc.sync.dma_start(out=c_mask[:], in_=mask[:, :])
            c_w1 = []
            for nm, t in (("A1", A1), ("B1", B1), ("C1", C1)):
                w = cpool.tile([IN_F, HID_F], BF16, tag=nm)
                nc.sync.dma_start(out=w[:], in_=t[:, :])
                c_w1.append(w)
            c_w2 = []
            for nm, t in (("A2", A2), ("B2", B2), ("C2", C2)):
                w = cpool.tile([HID_F, OUT_F], BF16, tag=nm)
                nc.sync.dma_start(out=w[:], in_=t[:, :])
                c_w2.append(w)
            c_b1 = cpool.tile([P, HID_F], F32, tag="b1b")
            nc.sync.dma_start(out=c_b1[:], in_=b1b[:, :])
            c_b2 = cpool.tile([P, OUT_F], F32, tag="b2b")
            nc.sync.dma_start(out=c_b2[:], in_=b2b[:, :])
            c_id = cpool.tile([P, P], F32, tag="ident")
            nc.sync.dma_start(out=c_id[:], in_=ident[:, :])
            c_idb = cpool.tile([P, P], BF16, tag="identb")
            nc.sync.dma_start(out=c_idb[:], in_=identb[:, :])

            # ---------------- phase 1: KAN layer 1 on the shard ----------------
            sc_p1, _ = nc.enter_named_scope("phase1", False)
            blocks = [(b * NB1, NB1) for b in range(96 // NB1)] + [(96, 1), (97, 1)]
            for (ga, nb) in blocks:
                w = nb * P
                xt = p1pool.tile([IN_F, NB1 * P], BF16, tag="xt")
                nc.sync.dma_start(out=xt[:, :w], in_=xT[:, ga * P:ga * P + w])
                x2 = p1pool.tile([IN_F, NB1 * P], BF16, tag="x2")
                nc.vector.tensor_tensor(out=x2[:, :w], in0=xt[:, :w], in1=xt[:, :w],
                                        op=mybir.AluOpType.mult)
                x3 = p1pool.tile([IN_F, NB1 * P], BF16, tag="x3")
                nc.vector.tensor_tensor(out=x3[:, :w], in0=x2[:, :w], in1=xt[:, :w],
                                        op=mybir.AluOpType.mult)
                ps = ppool.tile([P, NB1 * HID_F], F32, tag="k1")
                for gg in range(nb):
                    sl = slice(gg * P, (gg + 1) * P)
                    ol = slice(gg * HID_F, (gg + 1) * HID_F)
                    nc.tensor.matmul(out=ps[:, ol], lhsT=xt[:, sl], rhs=c_w1[0][:],
                                     start=True, stop=False)
                    nc.tensor.matmul(out=ps[:, ol], lhsT=x2[:, sl], rhs=c_w1[1][:],
                                     start=False, stop=False)
                    nc.tensor.matmul(out=ps[:, ol], lhsT=x3[:, sl], rhs=c_w1[2][:],
                                     start=False, stop=True)
                hb = p1pool.tile([P, NB1 * HID_F], F32, tag="hb")
                b1bc = c_b1[:, :].unsqueeze(1).to_broadcast([P, nb, HID_F])
                nc.vector.tensor_tensor(
                    out=hb[:, :nb * HID_F].rearrange("p (g f) -> p g f", g=nb),
                    in0=ps[:, :nb * HID_F].rearrange("p (g f) -> p g f", g=nb),
                    in1=b1bc, op=mybir.AluOpType.add)
                h1t = p1pool.tile([P, NB1 * HID_F], BF16, tag="h1t")
                if ga == 97:
                    nc.scalar.activation(out=h1t[:, :HID_F], in_=hb[:, :HID_F],
                                         func=mybir.ActivationFunctionType.Relu,
                                         scale=c_mask[:, 97:98])
                else:
                    nc.scalar.activation(out=h1t[:, :nb * HID_F],
                                         in_=hb[:, :nb * HID_F],
                                         func=mybir.ActivationFunctionType.Relu)
                nc.sync.dma_start(
                    out=h1_in[ga * P:(ga + nb) * P, :].rearrange(
                        "(g p) f -> p g f", p=P),
                    in_=h1t[:, :nb * HID_F].rearrange("p (g f) -> p g f", g=nb))
                if dump:
                    nc.sync.dma_start(
                        out=h1o[ga * P:(ga + nb) * P, :].rearrange(
                            "(g p) f -> p g f", p=P),
                        in_=h1t[:, :nb * HID_F].rearrange("p (g f) -> p g f", g=nb))
            nc.leave_named_scope("phase1", sc_p1, False)

            # ---------------- AllGather h1 ----------------
            sc_ag1, _ = nc.enter_named_scope("ag1", False)
            nc.gpsimd.collective_compute(
                "AllGather", mybir.AluOpType.bypass,
                replica_groups=[list(range(K))],
                ins=[h1_in[:, :]], outs=[h1_tbl[:, :]],
            )
            nc.leave_named_scope("ag1", sc_ag1, False)

            def gather_reduce(bt, table, F):
                """4 block gathers + strided tree reduce; returns [P, ng, F]
                view of the combined partial sums (in the color-0 block)."""
                ga, ng, Ds = bt["ga"], bt["ng"], bt["Ds"]
                dsum = sum(Ds)
                it = ipool.tile([P, 8 * 4 * RCAP], I16, tag="idx")
                ncols16 = 8 * ng * dsum
                nc.sync.dma_start(out=it[:, :ncols16],
                                  in_=idx[:, bt["icol"]:bt["icol"] + ncols16])
                # separate byte buffer per range block so the 4 queue-pairs
                # run concurrently and batch n+1 gathers overlap batch n
                # reduces; shared between phases via bitcast.
                views = []
                for R in range(NC_COL):
                    D = Ds[R]
                    nidx = P * ng * D
                    gt_raw = g1pool.tile([P, RCAP * HID_F], BF16, tag=f"g{R}")
                    gt = gt_raw[:, :] if table.dtype == BF16 else \
                        gt_raw[:, :].bitcast(F32)
                    nc.gpsimd.dma_gather(
                        out_ap=gt[:, :ng * D * F].rearrange(
                            "p (c f) -> p c f", c=ng * D),
                        in_ap=table[R * BLK:(R + 1) * BLK, :],
                        idxs_ap=it[:, 8 * ng * sum(Ds[:R]):
                                   8 * ng * sum(Ds[:R]) + nidx // 16],
                        num_idxs=nidx,
                        num_idxs_reg=nidx,
                        elem_size=F,
                        single_packet=False,
                        queue_num=R,
                    )
                    v = gt[:, :ng * D * F].rearrange("p (g x) -> p g x", g=ng)
                    cur = D
                    while cur > 1:
                        h = cur // 2
                        nc.vector.tensor_tensor(
                            out=v[:, :, :h * F], in0=v[:, :, :h * F],
                            in1=v[:, :, (cur - h) * F:cur * F],
                            op=mybir.AluOpType.add)
                        cur = cur - h
                    views.append(v[:, :, :F])
                acc = views[0]
                for R in range(1, NC_COL):
                    nc.vector.tensor_tensor(out=acc, in0=acc, in1=views[R],
                                            op=mybir.AluOpType.add)
                return acc

            # ---------------- phase 2: aggregate + KAN layer 2 ----------------
            # p-major gathers (verified path), then pipelined per-group PE
            # transposes feed a batched feature-major snT; KAN2 matmuls run on
            # unscaled sums and the per-target 1/deg powers are applied after
            # (scaling commutes through the elementwise powers).
            sc_p2, _ = nc.enter_named_scope("phase2", False)
            for bt in batches:
                ga, ng = bt["ga"], bt["ng"]
                ngP = ng * P
                acc = gather_reduce(bt, h1_tbl, HID_F)
                snT = wpool.tile([HID_F, NGMAX * P], BF16, tag="snT")
                for gg in range(ng):
                    pt = ppool2.tile([P, P], BF16, tag="tr")
                    nc.tensor.transpose(
                        out=pt[:],
                        in_=acc[:, gg:gg + 1, :].rearrange("p a f -> p (a f)"),
                        identity=c_idb[:])
                    nc.scalar.copy(out=snT[:, gg * P:(gg + 1) * P], in_=pt[:])
                q2b = wpool.tile([HID_F, NGMAX * P], BF16, tag="q2b")
                nc.vector.tensor_tensor(out=q2b[:, :ngP], in0=snT[:, :ngP],
                                        in1=snT[:, :ngP],
                                        op=mybir.AluOpType.mult)
                q3b = wpool.tile([HID_F, NGMAX * P], BF16, tag="q3b")
                nc.vector.tensor_tensor(out=q3b[:, :ngP], in0=q2b[:, :ngP],
                                        in1=snT[:, :ngP],
                                        op=mybir.AluOpType.mult)
                for gg in range(ng):
                    g = ga + gg
                    sl = slice(gg * P, (gg + 1) * P)
                    ps2 = ppool2.tile([P, 3 * OUT_F], F32, tag="k2")
                    nc.tensor.matmul(out=ps2[:, :OUT_F], lhsT=snT[:, sl],
                                     rhs=c_w2[0][:], start=True, stop=True)
                    nc.tensor.matmul(out=ps2[:, OUT_F:2 * OUT_F],
                                     lhsT=q2b[:, sl],
                                     rhs=c_w2[1][:], start=True, stop=True)
                    nc.tensor.matmul(out=ps2[:, 2 * OUT_F:3 * OUT_F],
                                     lhsT=q3b[:, sl],
                                     rhs=c_w2[2][:], start=True, stop=True)
                    u = wpool.tile([P, OUT_F], F32, tag="u")
                    nc.vector.tensor_scalar_mul(u[:], ps2[:, :OUT_F],
                                                c_dr[:, g:g + 1])
                    v = wpool.tile([P, OUT_F], F32, tag="v")
                    nc.vector.tensor_scalar_mul(v[:], ps2[:, OUT_F:2 * OUT_F],
                                                c_dr2[:, g:g + 1])
                    w = wpool.tile([P, OUT_F], F32, tag="w")
                    nc.vector.tensor_scalar_mul(w[:], ps2[:, 2 * OUT_F:],
                                                c_dr3[:, g:g + 1])
                    nc.vector.tensor_tensor(out=u[:], in0=u[:], in1=v[:],
                                            op=mybir.AluOpType.add)
                    nc.vector.tensor_tensor(out=u[:], in0=u[:], in1=w[:],
                                            op=mybir.AluOpType.add)
                    h2t = wpool.tile([P, OUT_F], F32, tag="h2t")
                    nc.vector.tensor_tensor(out=h2t[:], in0=u[:], in1=c_b2[:],
                                            op=mybir.AluOpType.add)
                    if g >= 97:
                        nc.vector.tensor_scalar_mul(h2t[:], h2t[:],
                                                    c_mask[:, g:g + 1])
                    nc.sync.dma_start(out=h2_in[g * P:(g + 1) * P, :],
                                      in_=h2t[:])
            nc.leave_named_scope("phase2", sc_p2, False)

            # ---------------- AllGather h2 ----------------
            sc_ag2, _ = nc.enter_named_scope("ag2", False)
            nc.gpsimd.collective_compute(
                "AllGather", mybir.AluOpType.bypass,
                replica_groups=[list(range(K))],
                ins=[h2_in[:, :]], outs=[h2_tbl[:, :]],
            )
            nc.leave_named_scope("ag2", sc_ag2, False)

            # ---------------- phase 3: aggregate + log_softmax ----------------
            sc_p3, _ = nc.enter_named_scope("phase3", False)
            tn_all = cpool.tile([P, G * OUT_F], F32, tag="tn_all")
            se_all = cpool.tile([P, G], F32, tag="se_all")
            for bt in batches:
                ga, ng = bt["ga"], bt["ng"]
                acc = gather_reduce(bt, h2_tbl, OUT_F)
                tnv = tn_all[:, ga * OUT_F:(ga + ng) * OUT_F].rearrange(
                    "p (g f) -> p g f", g=ng)
                drb = c_dr[:, ga:ga + ng].unsqueeze(2).to_broadcast([P, ng, OUT_F])
                nc.vector.tensor_tensor(out=tnv, in0=acc, in1=drb,
                                        op=mybir.AluOpType.mult)
                mxb = wpool.tile([P, NGMAX], F32, tag="mxb")
                nc.vector.tensor_reduce(out=mxb[:, :ng].unsqueeze(2), in_=tnv,
                                        axis=mybir.AxisListType.X,
                                        op=mybir.AluOpType.max)
                nc.vector.tensor_tensor(
                    out=tnv, in0=tnv,
                    in1=mxb[:, :ng].unsqueeze(2).to_broadcast([P, ng, OUT_F]),
                    op=mybir.AluOpType.subtract)
                et = wpool.tile([P, NGMAX * OUT_F], F32, tag="et")
                nc.scalar.activation(
                    out=et[:, :ng * OUT_F],
                    in_=tn_all[:, ga * OUT_F:(ga + ng) * OUT_F],
                    func=mybir.ActivationFunctionType.Exp)
                nc.vector.tensor_reduce(
                    out=se_all[:, ga:ga + ng].unsqueeze(2),
                    in_=et[:, :ng * OUT_F].rearrange("p (g f) -> p g f", g=ng),
                    axis=mybir.AxisListType.X, op=mybir.AluOpType.add)
            lse = cpool.tile([P, G], F32, tag="lse")
            nc.scalar.activation(out=lse[:], in_=se_all[:],
                                 func=mybir.ActivationFunctionType.Ln)
            for bt in batches:
                ga, ng = bt["ga"], bt["ng"]
                ot = wpool.tile([P, NGMAX * OUT_F], F32, tag="ot")
                nc.vector.tensor_tensor(
                    out=ot[:, :ng * OUT_F].rearrange("p (g f) -> p g f", g=ng),
                    in0=tn_all[:, ga * OUT_F:(ga + ng) * OUT_F].rearrange(
                        "p (g f) -> p g f", g=ng),
                    in1=lse[:, ga:ga + ng].unsqueeze(2).to_broadcast(
                        [P, ng, OUT_F]),
                    op=mybir.AluOpType.subtract)
                nc.sync.dma_start(
                    out=y[ga * P:(ga + ng) * P, :].rearrange(
                        "(g p) f -> p g f", p=P),
                    in_=ot[:, :ng * OUT_F].rearrange("p (g f) -> p g f", g=ng))
            nc.leave_named_scope("phase3", sc_p3, False)

    nc.compile()
    return nc


def kernel(x, edge_index, w1, b1, c1, w2, b2, c2):
    in_maps, meta = _host_prep(x, edge_index, w1, b1, c1, w2, b2, c2)
    nc = build_program(meta["batches"], meta["TOTC"])
    res = run_bass_kernel_spmd(nc, in_maps, core_ids=list(range(K)))
    out = np.empty((N_NODES, OUT_F), dtype=np.float32)
    core_of, j_of = meta["core_of_node"], meta["j_of_node"]
    for k in range(K):
        nodes_k = np.where(core_of == k)[0]
        out[nodes_k] = res.results[k]["y"][j_of[nodes_k]]
    return out


# revision 54
# speedup vs baseline: 1.8066x; 1.8066x over previous
"""KAN-GNN message passing on 8 TRN2 NeuronCores.

Strategy (data-parallel over nodes, per sharding hint):
 - Nodes are ranked by in-degree and dealt round-robin to the 8 cores, so
   every core holds 12500 targets with a near-identical degree profile.
 - Per core: KAN layer 1 on its node shard (3 fused matmuls: x, x^2, x^3
   against host-combined weights, bias via PSUM pre-init, relu+pad-mask in
   the ACT drain), then an AllGather of the bf16 h1 table.
 - Aggregation = one indirect-DMA gather per 128-target group: each target
   (partition) pulls its padded list of source rows side by side in the
   free dim, then a contiguous halving tree of DVE adds reduces the slots;
   scale by 1/deg, PE-transpose to put features on partitions, KAN layer 2,
   AllGather of h2, second gather/reduce, log_softmax.
 - All indices/permutations are precomputed on the host and baked into the
   (single, SPMD) program; per-core data goes in as input tensors.
"""
import numpy as np
import ml_dtypes

import concourse.bacc as bacc
import concourse.mybir as mybir
import concourse.tile as tile
import concourse.bass as bass
from concourse.bass_utils import run_bass_kernel_spmd

N_NODES = 100000
N_EDGES = 1600000
IN_F, HID_F, OUT_F = 128, 128, 64
K = 8               # cores
P = 128             # partitions / targets per group
J = 12544           # local nodes per core (98 * 128), 12500 real + 44 pad
G = J // P          # 98 groups
JREAL = N_NODES // K  # 12500
TBL = K * J         # 100352 rows in the all-gathered tables
PAD_POS = JREAL     # position (core 0, j=12500) -> guaranteed zero row

BF16 = mybir.dt.bfloat16
F32 = mybir.dt.float32
I32 = mybir.dt.int32


def _host_prep(x, edge_index, w1, b1, c1, w2, b2, c2):
    src = np.asarray(edge_index[0], dtype=np.int64)
    tgt = np.asarray(edge_index[1], dtype=np.int64)
    x = np.asarray(x, dtype=np.float32)

    deg = np.bincount(tgt, minlength=N_NODES)
    order = np.argsort(-deg, kind="stable")          # global degree rank -> node
    rank_of = np.empty(N_NODES, dtype=np.int64)
    rank_of[order] = np.arange(N_NODES)
    core_of_node = rank_of % K
    j_of_node = rank_of // K
    pos_of_node = core_of_node * J + j_of_node        # row in AG tables

    # per-core local degree [K, J]
    degs_kj = np.zeros((K, J), dtype=np.int64)
    degs_kj[core_of_node, j_of_node] = deg

    # group slot counts (shared across cores -> same program)
    Dg = degs_kj.reshape(K, G, P).max(axis=(0, 2))    # [G]
    Dg = np.maximum(Dg, 1).astype(np.int64)
    offs = np.concatenate([[0], np.cumsum(Dg)])       # [G+1]
    S = int(offs[-1])

    # slot table: idx_all[k, p, col] = table position of the d-th source of
    # local target (g*128+p) on core k; PAD_POS when d >= degree.
    idx_all = np.full((K, P, S), PAD_POS, dtype=np.int32)
    ek = core_of_node[tgt]
    ej = j_of_node[tgt]
    key = ek * J + ej
    eorder = np.argsort(key, kind="stable")
    skey = key[eorder]
    ukey, counts = np.unique(skey, return_counts=True)
    run_starts = np.concatenate([[0], np.cumsum(counts)[:-1]])
    d_in_run = np.arange(len(skey)) - np.repeat(run_starts, counts)
    ks = skey // J
    js = skey % J
    gs = js // P
    ps = js % P
    cols = offs[gs] + d_in_run
    idx_all[ks, ps, cols] = pos_of_node[src[eorder]].astype(np.int32)

    # per-core 1/deg  [K, P, G]  (0 for pad targets)
    with np.errstate(divide="ignore"):
        dr = 1.0 / np.maximum(degs_kj, 1).astype(np.float32)
    real = np.zeros((K, J), dtype=np.float32)
    real[:, :JREAL] = 1.0
    # reference divides by max(deg,1); deg-0 real targets get sum 0 -> 0 fine
    degrecip = (dr * np.where(real > 0, 1.0, 0.0)).reshape(K, G, P).transpose(0, 2, 1).copy()

    # pad-node mask [P, G] (same on every core)
    mask_j = (np.arange(J) < JREAL).astype(np.float32)
    mask_pg = mask_j.reshape(G, P).T.copy()

    # xT shards, bf16 [K][IN_F, J]
    xT = np.zeros((K, IN_F, J), dtype=ml_dtypes.bfloat16)
    for k in range(K):
        nodes_k = order[np.arange(JREAL) * K + k]
        xT[k, :, :JREAL] = x[nodes_k].T.astype(ml_dtypes.bfloat16)

    # fused KAN weights
    A1 = (w1 + 0.1 * c1[:, :, 0]).astype(ml_dtypes.bfloat16)
    B1 = (0.1 * c1[:, :, 1]).astype(ml_dtypes.bfloat16)
    C1 = (0.1 * c1[:, :, 2]).astype(ml_dtypes.bfloat16)
    A2 = (w2 + 0.1 * c2[:, :, 0]).astype(ml_dtypes.bfloat16)
    B2 = (0.1 * c2[:, :, 1]).astype(ml_dtypes.bfloat16)
    C2 = (0.1 * c2[:, :, 2]).astype(ml_dtypes.bfloat16)
    b1b = np.tile(np.asarray(b1, np.float32)[None, :], (P, 1))
    b2b = np.tile(np.asarray(b2, np.float32)[None, :], (P, 1))
    ident = np.eye(P, dtype=np.float32)

    in_maps = []
    for k in range(K):
        in_maps.append({
            "xT": xT[k],
            "idx": idx_all[k],
            "degrecip": degrecip[k],
            "mask": mask_pg,
            "A1": A1, "B1": B1, "C1": C1,
            "A2": A2, "B2": B2, "C2": C2,
            "b1b": b1b, "b2b": b2b, "ident": ident,
        })
    meta = {"Dg": Dg, "offs": offs, "S": S, "order": order}
    return in_maps, meta


def _tree_reduce(nc, tiles_ap, D, F):
    """In-place halving tree over D slots of width F. Returns slice [P, F]."""
    cur = D
    while cur > 1:
        h = cur // 2
        nc.vector.tensor_tensor(
            out=tiles_ap[:, : h * F],
            in0=tiles_ap[:, : h * F],
            in1=tiles_ap[:, (cur - h) * F: cur * F],
            op=mybir.AluOpType.add,
        )
        cur = cur - h
    return tiles_ap[:, :F]


def build_program(Dg, offs, S, dump=False):
    nc = bacc.Bacc("TRN2", target_bir_lowering=False, debug=False, num_devices=K,
                   dynamic_dma_scratch_size=131072)

    xT = nc.dram_tensor("xT", [IN_F, J], BF16, kind="ExternalInput")
    idx = nc.dram_tensor("idx", [P, S], I32, kind="ExternalInput")
    degrecip = nc.dram_tensor("degrecip", [P, G], F32, kind="ExternalInput")
    mask = nc.dram_tensor("mask", [P, G], F32, kind="ExternalInput")
    A1 = nc.dram_tensor("A1", [IN_F, HID_F], BF16, kind="ExternalInput")
    B1 = nc.dram_tensor("B1", [IN_F, HID_F], BF16, kind="ExternalInput")
    C1 = nc.dram_tensor("C1", [IN_F, HID_F], BF16, kind="ExternalInput")
    A2 = nc.dram_tensor("A2", [HID_F, OUT_F], BF16, kind="ExternalInput")
    B2 = nc.dram_tensor("B2", [HID_F, OUT_F], BF16, kind="ExternalInput")
    C2 = nc.dram_tensor("C2", [HID_F, OUT_F], BF16, kind="ExternalInput")
    b1b = nc.dram_tensor("b1b", [P, HID_F], F32, kind="ExternalInput")
    b2b = nc.dram_tensor("b2b", [P, OUT_F], F32, kind="ExternalInput")
    ident = nc.dram_tensor("ident", [P, P], F32, kind="ExternalInput")
    y = nc.dram_tensor("y", [J, OUT_F], F32, kind="ExternalOutput")
    if dump:
        h1o = nc.dram_tensor("h1o", [J, HID_F], BF16, kind="ExternalOutput")
        sno = nc.dram_tensor("sno", [J, HID_F], F32, kind="ExternalOutput")

    h1_in = nc.dram_tensor("h1_in", [J, HID_F], BF16, kind="Internal")
    h1_tbl = nc.dram_tensor("h1_tbl", [TBL, HID_F], BF16, kind="Internal",
                            addr_space="Shared")
    h2_in = nc.dram_tensor("h2_in", [J, OUT_F], BF16, kind="Internal")
    h2_tbl = nc.dram_tensor("h2_tbl", [TBL, OUT_F], BF16, kind="Internal",
                            addr_space="Shared")

    Dmax = int(max(Dg))

    with tile.TileContext(nc) as tc:
        with (
            tc.tile_pool(name="consts", bufs=1) as cpool,
            tc.tile_pool(name="work", bufs=3) as wpool,
            tc.tile_pool(name="gather", bufs=3) as gpool,
            tc.tile_pool(name="psum", bufs=2, space="PSUM") as ppool,
        ):
            # load constants
            c_idx = cpool.tile([P, S], I32, tag="idx")
            nc.sync.dma_start(out=c_idx[:], in_=idx[:, :])
            c_dr = cpool.tile([P, G], F32, tag="dr")
            nc.sync.dma_start(out=c_dr[:], in_=degrecip[:, :])
            c_mask = cpool.tile([P, G], F32, tag="mask")
            nc.sync.dma_start(out=c_mask[:], in_=mask[:, :])
            c_w1 = []
            for nm, t in (("A1", A1), ("B1", B1), ("C1", C1)):
                w = cpool.tile([IN_F, HID_F], BF16, tag=nm)
                nc.sync.dma_start(out=w[:], in_=t[:, :])
                c_w1.append(w)
            c_w2 = []
            for nm, t in (("A2", A2), ("B2", B2), ("C2", C2)):
                w = cpool.tile([HID_F, OUT_F], BF16, tag=nm)
                nc.sync.dma_start(out=w[:], in_=t[:, :])
                c_w2.append(w)
            c_b1 = cpool.tile([P, HID_F], F32, tag="b1b")
            nc.sync.dma_start(out=c_b1[:], in_=b1b[:, :])
            c_b2 = cpool.tile([P, OUT_F], F32, tag="b2b")
            nc.sync.dma_start(out=c_b2[:], in_=b2b[:, :])
            c_id = cpool.tile([P, P], F32, tag="ident")
            nc.sync.dma_start(out=c_id[:], in_=ident[:, :])

            # ---------------- phase 1: KAN layer 1 on the shard ----------------
            for g in range(G):
                xt = wpool.tile([IN_F, P], BF16, tag="xt")
                nc.sync.dma_start(out=xt[:], in_=xT[:, g * P:(g + 1) * P])
                x2 = wpool.tile([IN_F, P], BF16, tag="x2")
                nc.vector.tensor_tensor(out=x2[:], in0=xt[:], in1=xt[:],
                                        op=mybir.AluOpType.mult)
                x3 = wpool.tile([IN_F, P], BF16, tag="x3")
                nc.vector.tensor_tensor(out=x3[:], in0=x2[:], in1=xt[:],
                                        op=mybir.AluOpType.mult)
                ps = ppool.tile([P, HID_F], F32, tag="k1")
                nc.tensor.matmul(out=ps[:], lhsT=xt[:], rhs=c_w1[0][:],
                                 start=True, stop=False)
                nc.tensor.matmul(out=ps[:], lhsT=x2[:], rhs=c_w1[1][:],
                                 start=False, stop=False)
                nc.tensor.matmul(out=ps[:], lhsT=x3[:], rhs=c_w1[2][:],
                                 start=False, stop=True)
                hb = wpool.tile([P, HID_F], F32, tag="hb")
                nc.vector.tensor_tensor(out=hb[:], in0=ps[:], in1=c_b1[:],
                                        op=mybir.AluOpType.add)
                h1t = wpool.tile([P, HID_F], BF16, tag="h1t")
                nc.scalar.activation(out=h1t[:], in_=hb[:],
                                     func=mybir.ActivationFunctionType.Relu,
                                     scale=c_mask[:, g:g + 1])
                nc.sync.dma_start(out=h1_in[g * P:(g + 1) * P, :], in_=h1t[:])
                if dump:
                    nc.sync.dma_start(out=h1o[g * P:(g + 1) * P, :], in_=h1t[:])

            # ---------------- AllGather h1 ----------------
            nc.gpsimd.collective_compute(
                "AllGather", mybir.AluOpType.bypass,
                replica_groups=[list(range(K))],
                ins=[h1_in[:, :]], outs=[h1_tbl[:, :]],
            )

            # ---------------- phase 2: aggregate + KAN layer 2 ----------------
            for g in range(G):
                D = int(Dg[g])
                off = int(offs[g])
                gt = gpool.tile([P, Dmax * HID_F], BF16, tag="g1")
                for d in range(D):
                    nc.gpsimd.indirect_dma_start(
                        out=gt[:, d * HID_F:(d + 1) * HID_F],
                        out_offset=None,
                        in_=h1_tbl[:, :],
                        in_offset=bass.IndirectOffsetOnAxis(
                            ap=c_idx[:, off + d:off + d + 1], axis=0),
                    )
                s1 = _tree_reduce(nc, gt, D, HID_F)
                sn = wpool.tile([P, HID_F], F32, tag="sn")
                nc.vector.tensor_scalar_mul(sn[:], s1, c_dr[:, g:g + 1])
                if dump:
                    nc.sync.dma_start(out=sno[g * P:(g + 1) * P, :], in_=sn[:])
                pt = ppool.tile([P, P], F32, tag="tr")
                nc.tensor.transpose(out=pt[:], in_=sn[:], identity=c_id[:])
                hT = wpool.tile([HID_F, P], BF16, tag="hT")
                nc.scalar.copy(out=hT[:], in_=pt[:])
                q2 = wpool.tile([HID_F, P], BF16, tag="q2")
                nc.vector.tensor_tensor(out=q2[:], in0=hT[:], in1=hT[:],
                                        op=mybir.AluOpType.mult)
                q3 = wpool.tile([HID_F, P], BF16, tag="q3")
                nc.vector.tensor_tensor(out=q3[:], in0=q2[:], in1=hT[:],
                                        op=mybir.AluOpType.mult)
                ps2 = ppool.tile([P, OUT_F], F32, tag="k2")
                nc.tensor.matmul(out=ps2[:], lhsT=hT[:], rhs=c_w2[0][:],
                                 start=True, stop=False)
                nc.tensor.matmul(out=ps2[:], lhsT=q2[:], rhs=c_w2[1][:],
                                 start=False, stop=False)
                nc.tensor.matmul(out=ps2[:], lhsT=q3[:], rhs=c_w2[2][:],
                                 start=False, stop=True)
                hb2 = wpool.tile([P, OUT_F], F32, tag="hb2")
                nc.vector.tensor_tensor(out=hb2[:], in0=ps2[:], in1=c_b2[:],
                                        op=mybir.AluOpType.add)
                h2t = wpool.tile([P, OUT_F], BF16, tag="h2t")
                nc.scalar.activation(out=h2t[:], in_=hb2[:],
                                     func=mybir.ActivationFunctionType.Copy,
                                     scale=c_mask[:, g:g + 1])
                nc.sync.dma_start(out=h2_in[g * P:(g + 1) * P, :], in_=h2t[:])

            # ---------------- AllGather h2 ----------------
            nc.gpsimd.collective_compute(
                "AllGather", mybir.AluOpType.bypass,
                replica_groups=[list(range(K))],
                ins=[h2_in[:, :]], outs=[h2_tbl[:, :]],
            )

            # ---------------- phase 3: aggregate + log_softmax ----------------
            for g in range(G):
                D = int(Dg[g])
                off = int(offs[g])
                gt = gpool.tile([P, Dmax * OUT_F], BF16, tag="g2")
                for d in range(D):
                    nc.gpsimd.indirect_dma_start(
                        out=gt[:, d * OUT_F:(d + 1) * OUT_F],
                        out_offset=None,
                        in_=h2_tbl[:, :],
                        in_offset=bass.IndirectOffsetOnAxis(
                            ap=c_idx[:, off + d:off + d + 1], axis=0),
                    )
                s2 = _tree_reduce(nc, gt, D, OUT_F)
                tn = wpool.tile([P, OUT_F], F32, tag="tn")
                nc.vector.tensor_scalar_mul(tn[:], s2, c_dr[:, g:g + 1])
                mx = wpool.tile([P, 1], F32, tag="mx")
                nc.vector.tensor_reduce(out=mx[:], in_=tn[:],
                                        axis=mybir.AxisListType.X,
                                        op=mybir.AluOpType.max)
                nmx = wpool.tile([P, 1], F32, tag="nmx")
                nc.vector.tensor_scalar_mul(nmx[:], mx[:], -1.0)
                et = wpool.tile([P, OUT_F], F32, tag="et")
                se = wpool.tile([P, 1], F32, tag="se")
                nc.scalar.activation(out=et[:], in_=tn[:],
                                     func=mybir.ActivationFunctionType.Exp,
                                     bias=nmx[:, :1], scale=1.0,
                                     accum_out=se[:, :1])
                lse = wpool.tile([P, 1], F32, tag="lse")
                nc.scalar.activation(out=lse[:], in_=se[:],
                                     func=mybir.ActivationFunctionType.Ln)
                ot = wpool.tile([P, OUT_F], F32, tag="ot")
                nc.vector.tensor_scalar(ot[:], tn[:], nmx[:, :1], lse[:, :1],
                                        mybir.AluOpType.add,
                                        mybir.AluOpType.subtract)
                nc.sync.dma_start(out=y[g * P:(g + 1) * P, :], in_=ot[:])

    nc.compile()
    return nc


def kernel(x, edge_index, w1, b1, c1, w2, b2, c2):
    in_maps, meta = _host_prep(x, edge_index, w1, b1, c1, w2, b2, c2)
    nc = build_program(meta["Dg"], meta["offs"], meta["S"])
    res = run_bass_kernel_spmd(nc, in_maps, core_ids=list(range(K)))
    order = meta["order"]
    out = np.empty((N_NODES, OUT_F), dtype=np.float32)
    jr = np.arange(JREAL)
    for k in range(K):
        out[order[jr * K + k]] = res.results[k]["y"][:JREAL]
    return out


# revision 55
# speedup vs baseline: 1.8127x; 1.0034x over previous
"""KAN-GNN message passing on 8 TRN2 NeuronCores.

Strategy (data-parallel over nodes, per sharding hint):
 - Nodes are ranked by in-degree and dealt round-robin to the 8 cores, so
   every core holds 12500 targets with a near-identical degree profile.
 - Per core: KAN layer 1 on its node shard (3 fused matmuls: x, x^2, x^3
   against host-combined weights, bias via PSUM pre-init, relu+pad-mask in
   the ACT drain), then an AllGather of the bf16 h1 table.
 - Aggregation = one indirect-DMA gather per 128-target group: each target
   (partition) pulls its padded list of source rows side by side in the
   free dim, then a contiguous halving tree of DVE adds reduces the slots;
   scale by 1/deg, PE-transpose to put features on partitions, KAN layer 2,
   AllGather of h2, second gather/reduce, log_softmax.
 - All indices/permutations are precomputed on the host and baked into the
   (single, SPMD) program; per-core data goes in as input tensors.
"""
import numpy as np
import ml_dtypes

import concourse.bacc as bacc
import concourse.mybir as mybir
import concourse.tile as tile
import concourse.bass as bass
from concourse.bass_utils import run_bass_kernel_spmd

N_NODES = 100000
N_EDGES = 1600000
IN_F, HID_F, OUT_F = 128, 128, 64
K = 8               # cores
P = 128             # partitions / targets per group
J = 12544           # local nodes per core (98 * 128), 12500 real + 44 pad
G = J // P          # 98 groups
JREAL = N_NODES // K  # 12500
TBL = K * J         # 100352 rows in the all-gathered tables
PAD_POS = JREAL     # position (core 0, j=12500) -> guaranteed zero row

BF16 = mybir.dt.bfloat16
F32 = mybir.dt.float32
I32 = mybir.dt.int32


def _host_prep(x, edge_index, w1, b1, c1, w2, b2, c2):
    src = np.asarray(edge_index[0], dtype=np.int64)
    tgt = np.asarray(edge_index[1], dtype=np.int64)
    x = np.asarray(x, dtype=np.float32)

    deg = np.bincount(tgt, minlength=N_NODES)
    order = np.argsort(-deg, kind="stable")          # global degree rank -> node
    rank_of = np.empty(N_NODES, dtype=np.int64)
    rank_of[order] = np.arange(N_NODES)
    core_of_node = rank_of % K
    j_of_node = rank_of // K
    pos_of_node = core_of_node * J + j_of_node        # row in AG tables

    # per-core local degree [K, J]
    degs_kj = np.zeros((K, J), dtype=np.int64)
    degs_kj[core_of_node, j_of_node] = deg

    # group slot counts (shared across cores -> same program)
    Dg = degs_kj.reshape(K, G, P).max(axis=(0, 2))    # [G]
    Dg = np.maximum(Dg, 1).astype(np.int64)
    offs = np.concatenate([[0], np.cumsum(Dg)])       # [G+1]
    S = int(offs[-1])

    # slot table: idx_all[k, p, col] = table position of the d-th source of
    # local target (g*128+p) on core k; PAD_POS when d >= degree.
    idx_all = np.full((K, P, S), PAD_POS, dtype=np.int32)
    ek = core_of_node[tgt]
    ej = j_of_node[tgt]
    key = ek * J + ej
    eorder = np.argsort(key, kind="stable")
    skey = key[eorder]
    ukey, counts = np.unique(skey, return_counts=True)
    run_starts = np.concatenate([[0], np.cumsum(counts)[:-1]])
    d_in_run = np.arange(len(skey)) - np.repeat(run_starts, counts)
    ks = skey // J
    js = skey % J
    gs = js // P
    ps = js % P
    cols = offs[gs] + d_in_run
    idx_all[ks, ps, cols] = pos_of_node[src[eorder]].astype(np.int32)

    # per-core 1/deg  [K, P, G]  (0 for pad targets)
    with np.errstate(divide="ignore"):
        dr = 1.0 / np.maximum(degs_kj, 1).astype(np.float32)
    real = np.zeros((K, J), dtype=np.float32)
    real[:, :JREAL] = 1.0
    # reference divides by max(deg,1); deg-0 real targets get sum 0 -> 0 fine
    degrecip = (dr * np.where(real > 0, 1.0, 0.0)).reshape(K, G, P).transpose(0, 2, 1).copy()

    # pad-node mask [P, G] (same on every core)
    mask_j = (np.arange(J) < JREAL).astype(np.float32)
    mask_pg = mask_j.reshape(G, P).T.copy()

    # xT shards, bf16 [K][IN_F, J]
    xT = np.zeros((K, IN_F, J), dtype=ml_dtypes.bfloat16)
    for k in range(K):
        nodes_k = order[np.arange(JREAL) * K + k]
        xT[k, :, :JREAL] = x[nodes_k].T.astype(ml_dtypes.bfloat16)

    # fused KAN weights
    A1 = (w1 + 0.1 * c1[:, :, 0]).astype(ml_dtypes.bfloat16)
    B1 = (0.1 * c1[:, :, 1]).astype(ml_dtypes.bfloat16)
    C1 = (0.1 * c1[:, :, 2]).astype(ml_dtypes.bfloat16)
    A2 = (w2 + 0.1 * c2[:, :, 0]).astype(ml_dtypes.bfloat16)
    B2 = (0.1 * c2[:, :, 1]).astype(ml_dtypes.bfloat16)
    C2 = (0.1 * c2[:, :, 2]).astype(ml_dtypes.bfloat16)
    b1b = np.tile(np.asarray(b1, np.float32)[None, :], (P, 1))
    b2b = np.tile(np.asarray(b2, np.float32)[None, :], (P, 1))
    ident = np.eye(P, dtype=np.float32)

    in_maps = []
    for k in range(K):
        in_maps.append({
            "xT": xT[k],
            "idx": idx_all[k],
            "degrecip": degrecip[k],
            "mask": mask_pg,
            "A1": A1, "B1": B1, "C1": C1,
            "A2": A2, "B2": B2, "C2": C2,
            "b1b": b1b, "b2b": b2b, "ident": ident,
        })
    meta = {"Dg": Dg, "offs": offs, "S": S, "order": order}
    return in_maps, meta


def _tree_reduce(nc, tiles_ap, D, F):
    """In-place halving tree over D slots of width F. Returns slice [P, F]."""
    cur = D
    while cur > 1:
        h = cur // 2
        nc.vector.tensor_tensor(
            out=tiles_ap[:, : h * F],
            in0=tiles_ap[:, : h * F],
            in1=tiles_ap[:, (cur - h) * F: cur * F],
            op=mybir.AluOpType.add,
        )
        cur = cur - h
    return tiles_ap[:, :F]


def build_program(Dg, offs, S, dump=False):
    nc = bacc.Bacc("TRN2", target_bir_lowering=False, debug=False, num_devices=K,
                   dynamic_dma_scratch_size=131072)

    xT = nc.dram_tensor("xT", [IN_F, J], BF16, kind="ExternalInput")
    idx = nc.dram_tensor("idx", [P, S], I32, kind="ExternalInput")
    degrecip = nc.dram_tensor("degrecip", [P, G], F32, kind="ExternalInput")
    mask = nc.dram_tensor("mask", [P, G], F32, kind="ExternalInput")
    A1 = nc.dram_tensor("A1", [IN_F, HID_F], BF16, kind="ExternalInput")
    B1 = nc.dram_tensor("B1", [IN_F, HID_F], BF16, kind="ExternalInput")
    C1 = nc.dram_tensor("C1", [IN_F, HID_F], BF16, kind="ExternalInput")
    A2 = nc.dram_tensor("A2", [HID_F, OUT_F], BF16, kind="ExternalInput")
    B2 = nc.dram_tensor("B2", [HID_F, OUT_F], BF16, kind="ExternalInput")
    C2 = nc.dram_tensor("C2", [HID_F, OUT_F], BF16, kind="ExternalInput")
    b1b = nc.dram_tensor("b1b", [P, HID_F], F32, kind="ExternalInput")
    b2b = nc.dram_tensor("b2b", [P, OUT_F], F32, kind="ExternalInput")
    ident = nc.dram_tensor("ident", [P, P], F32, kind="ExternalInput")
    y = nc.dram_tensor("y", [J, OUT_F], F32, kind="ExternalOutput")
    if dump:
        h1o = nc.dram_tensor("h1o", [J, HID_F], BF16, kind="ExternalOutput")
        sno = nc.dram_tensor("sno", [J, HID_F], F32, kind="ExternalOutput")

    h1_in = nc.dram_tensor("h1_in", [J, HID_F], BF16, kind="Internal")
    h1_tbl = nc.dram_tensor("h1_tbl", [TBL, HID_F], BF16, kind="Internal",
                            addr_space="Shared")
    h2_in = nc.dram_tensor("h2_in", [J, OUT_F], BF16, kind="Internal")
    h2_tbl = nc.dram_tensor("h2_tbl", [TBL, OUT_F], BF16, kind="Internal",
                            addr_space="Shared")

    Dmax = int(max(Dg))

    with tile.TileContext(nc) as tc:
        with (
            tc.tile_pool(name="consts", bufs=1) as cpool,
            tc.tile_pool(name="work", bufs=3) as wpool,
            tc.tile_pool(name="gather", bufs=3) as gpool,
            tc.tile_pool(name="psum", bufs=2, space="PSUM") as ppool,
        ):
            # load constants
            c_idx = cpool.tile([P, S], I32, tag="idx")
            nc.sync.dma_start(out=c_idx[:], in_=idx[:, :])
            c_dr = cpool.tile([P, G], F32, tag="dr")
            nc.sync.dma_start(out=c_dr[:], in_=degrecip[:, :])
            c_mask = cpool.tile([P, G], F32, tag="mask")
            nc.sync.dma_start(out=c_mask[:], in_=mask[:, :])
            c_w1 = []
            for nm, t in (("A1", A1), ("B1", B1), ("C1", C1)):
                w = cpool.tile([IN_F, HID_F], BF16, tag=nm)
                nc.sync.dma_start(out=w[:], in_=t[:, :])
                c_w1.append(w)
            c_w2 = []
            for nm, t in (("A2", A2), ("B2", B2), ("C2", C2)):
                w = cpool.tile([HID_F, OUT_F], BF16, tag=nm)
                nc.sync.dma_start(out=w[:], in_=t[:, :])
                c_w2.append(w)
            c_b1 = cpool.tile([P, HID_F], F32, tag="b1b")
            nc.sync.dma_start(out=c_b1[:], in_=b1b[:, :])
            c_b2 = cpool.tile([P, OUT_F], F32, tag="b2b")
            nc.sync.dma_start(out=c_b2[:], in_=b2b[:, :])
            c_id = cpool.tile([P, P], F32, tag="ident")
            nc.sync.dma_start(out=c_id[:], in_=ident[:, :])

            # ---------------- phase 1: KAN layer 1 on the shard ----------------
            for g in range(G):
                xt = wpool.tile([IN_F, P], BF16, tag="xt")
                nc.sync.dma_start(out=xt[:], in_=xT[:, g * P:(g + 1) * P])
                x2 = wpool.tile([IN_F, P], BF16, tag="x2")
                nc.vector.tensor_tensor(out=x2[:], in0=xt[:], in1=xt[:],
                                        op=mybir.AluOpType.mult)
                x3 = wpool.tile([IN_F, P], BF16, tag="x3")
                nc.vector.tensor_tensor(out=x3[:], in0=x2[:], in1=xt[:],
                                        op=mybir.AluOpType.mult)
                ps = ppool.tile([P, HID_F], F32, tag="k1")
                nc.tensor.matmul(out=ps[:], lhsT=xt[:], rhs=c_w1[0][:],
                                 start=True, stop=False)
                nc.tensor.matmul(out=ps[:], lhsT=x2[:], rhs=c_w1[1][:],
                                 start=False, stop=False)
                nc.tensor.matmul(out=ps[:], lhsT=x3[:], rhs=c_w1[2][:],
                                 start=False, stop=True)
                hb = wpool.tile([P, HID_F], F32, tag="hb")
                nc.vector.tensor_tensor(out=hb[:], in0=ps[:], in1=c_b1[:],
                                        op=mybir.AluOpType.add)
                h1t = wpool.tile([P, HID_F], BF16, tag="h1t")
                nc.scalar.activation(out=h1t[:], in_=hb[:],
                                     func=mybir.ActivationFunctionType.Relu,
                                     scale=c_mask[:, g:g + 1])
                nc.sync.dma_start(out=h1_in[g * P:(g + 1) * P, :], in_=h1t[:])
                if dump:
                    nc.sync.dma_start(out=h1o[g * P:(g + 1) * P, :], in_=h1t[:])

            # ---------------- AllGather h1 ----------------
            nc.gpsimd.collective_compute(
                "AllGather", mybir.AluOpType.bypass,
                replica_groups=[list(range(K))],
                ins=[h1_in[:, :]], outs=[h1_tbl[:, :]],
            )

            # ---------------- phase 2: aggregate + KAN layer 2 ----------------
            for g in range(G):
                D = int(Dg[g])
                off = int(offs[g])
                gt = gpool.tile([P, Dmax * HID_F], BF16, tag="g1")
                for d in range(D):
                    nc.gpsimd.indirect_dma_start(
                        out=gt[:, d * HID_F:(d + 1) * HID_F],
                        out_offset=None,
                        in_=h1_tbl[:, :],
                        in_offset=bass.IndirectOffsetOnAxis(
                            ap=c_idx[:, off + d:off + d + 1], axis=0),
                    )
                s1 = _tree_reduce(nc, gt, D, HID_F)
                sn = wpool.tile([P, HID_F], F32, tag="sn")
                nc.vector.tensor_scalar_mul(sn[:], s1, c_dr[:, g:g + 1])
                if dump:
                    nc.sync.dma_start(out=sno[g * P:(g + 1) * P, :], in_=sn[:])
                pt = ppool.tile([P, P], F32, tag="tr")
                nc.tensor.transpose(out=pt[:], in_=sn[:], identity=c_id[:])
                hT = wpool.tile([HID_F, P], BF16, tag="hT")
                nc.scalar.copy(out=hT[:], in_=pt[:])
                q2 = wpool.tile([HID_F, P], BF16, tag="q2")
                nc.vector.tensor_tensor(out=q2[:], in0=hT[:], in1=hT[:],
                                        op=mybir.AluOpType.mult)
                q3 = wpool.tile([HID_F, P], BF16, tag="q3")
                nc.vector.tensor_tensor(out=q3[:], in0=q2[:], in1=hT[:],
                                        op=mybir.AluOpType.mult)
                ps2 = ppool.tile([P, OUT_F], F32, tag="k2")
                nc.tensor.matmul(out=ps2[:], lhsT=hT[:], rhs=c_w2[0][:],
                                 start=True, stop=False)
                nc.tensor.matmul(out=ps2[:], lhsT=q2[:], rhs=c_w2[1][:],
                                 start=False, stop=False)
                nc.tensor.matmul(out=ps2[:], lhsT=q3[:], rhs=c_w2[2][:],
                                 start=False, stop=True)
                hb2 = wpool.tile([P, OUT_F], F32, tag="hb2")
                nc.vector.tensor_tensor(out=hb2[:], in0=ps2[:], in1=c_b2[:],
                                        op=mybir.AluOpType.add)
                h2t = wpool.tile([P, OUT_F], BF16, tag="h2t")
                nc.scalar.activation(out=h2t[:], in_=hb2[:],
                                     func=mybir.ActivationFunctionType.Copy,
                                     scale=c_mask[:, g:g + 1])
                nc.sync.dma_start(out=h2_in[g * P:(g + 1) * P, :], in_=h2t[:])

            # ---------------- AllGather h2 ----------------
            nc.gpsimd.collective_compute(
                "AllGather", mybir.AluOpType.bypass,
                replica_groups=[list(range(K))],
                ins=[h2_in[:, :]], outs=[h2_tbl[:, :]],
            )

            # ---------------- phase 3: aggregate + log_softmax ----------------
            # all Exp activations run back-to-back, then one Ln over the
            # accumulated sums: the ACT Exp/Ln tables each load exactly once
            # (the per-group Exp/Ln interleave costs ~1.3us per table swap).
            tn_all = cpool.tile([P, G * OUT_F], F32, tag="tn_all")
            nmx_all = cpool.tile([P, G], F32, tag="nmx_all")
            se_all = cpool.tile([P, G], F32, tag="se_all")
            for g in range(G):
                D = int(Dg[g])
                off = int(offs[g])
                gt = gpool.tile([P, Dmax * OUT_F], BF16, tag="g2")
                for d in range(D):
                    nc.gpsimd.indirect_dma_start(
                        out=gt[:, d * OUT_F:(d + 1) * OUT_F],
                        out_offset=None,
                        in_=h2_tbl[:, :],
                        in_offset=bass.IndirectOffsetOnAxis(
                            ap=c_idx[:, off + d:off + d + 1], axis=0),
                    )
                s2 = _tree_reduce(nc, gt, D, OUT_F)
                tn = tn_all[:, g * OUT_F:(g + 1) * OUT_F]
                nc.vector.tensor_scalar_mul(tn, s2, c_dr[:, g:g + 1])
                mx = wpool.tile([P, 1], F32, tag="mx")
                nc.vector.tensor_reduce(out=mx[:], in_=tn,
                                        axis=mybir.AxisListType.X,
                                        op=mybir.AluOpType.max)
                nc.vector.tensor_scalar_mul(nmx_all[:, g:g + 1], mx[:], -1.0)
                et = wpool.tile([P, OUT_F], F32, tag="et")
                nc.scalar.activation(out=et[:], in_=tn,
                                     func=mybir.ActivationFunctionType.Exp,
                                     bias=nmx_all[:, g:g + 1], scale=1.0,
                                     accum_out=se_all[:, g:g + 1])
            lse_all = cpool.tile([P, G], F32, tag="lse_all")
            nc.scalar.activation(out=lse_all[:], in_=se_all[:],
                                 func=mybir.ActivationFunctionType.Ln)
            for g in range(G):
                ot = wpool.tile([P, OUT_F], F32, tag="ot")
                nc.vector.tensor_scalar(ot[:], tn_all[:, g * OUT_F:(g + 1) * OUT_F],
                                        nmx_all[:, g:g + 1],
                                        lse_all[:, g:g + 1],
                                        mybir.AluOpType.add,
                                        mybir.AluOpType.subtract)
                nc.sync.dma_start(out=y[g * P:(g + 1) * P, :], in_=ot[:])

    nc.compile()
    return nc


def kernel(x, edge_index, w1, b1, c1, w2, b2, c2):
    in_maps, meta = _host_prep(x, edge_index, w1, b1, c1, w2, b2, c2)
    nc = build_program(meta["Dg"], meta["offs"], meta["S"])
    res = run_bass_kernel_spmd(nc, in_maps, core_ids=list(range(K)))
    order = meta["order"]
    out = np.empty((N_NODES, OUT_F), dtype=np.float32)
    jr = np.arange(JREAL)
    for k in range(K):
        out[order[jr * K + k]] = res.results[k]["y"][:JREAL]
    return out
